# revision 65
# baseline (speedup 1.0000x reference)
"""Multi-head attention (16 heads, RoPE, causal) for Trainium2, 8 NeuronCores.

Sharding: data-parallel over batch (2) x tensor-parallel over head groups (4),
one (batch, head-group-of-4) pair per core. Each core computes its 4 heads'
attention feature-major and a partial output projection outT [1024, 2048] in
fp16; the host sums the 4 partials per batch and transposes back.

Key design points (vs a straightforward bf16 kernel):
  - Q/K projections run in fp8e4 DoubleRow perf mode (2 contraction planes
    per matmul, 0.5 cycles/col = 4x bf16 MAC rate). RoPE's pair-swap is
    precomputed as a second projection with host-shuffled weight columns, so
    on-chip RoPE is just rot = cos*P + srot*Pswap: two PSUM multiplies and
    one add on DVE per tile, nothing on the scalar engine.
  - V projection is fp8 DoubleRow with first-order error compensation:
    V = x8@wv8 + x8@dwv8 + dx8@wv8 (residuals quantized in the same scale
    domain) - more accurate than a bf16 matmul at under half the cycles,
    and it removes the 4MB bf16 x load entirely.
  - The S^T matmul stays bf16 with K=64 contraction: at half array density
    it does not trip the PE power governor (fp8 S drains the power-credit
    pool and throttles the whole attention phase to a 50% duty cycle).
  - All fp8/bf16 scale factors cancel inside the exp() activation scale
    immediate; the rope tables are plain cos/srot, loaded as [32, L] and
    replicated to 128 partitions with two doubling DVE copies each.
  - Scalar (ACT) engine runs exp() only (~84us of 1/cycle/partition work,
    the critical resource); softmax denominators come from a ones-column in
    V_aug; 1/z is broadcast on gpsimd with the yt scale deferred one head
    so DVE never stalls on the broadcast latency.
  - Inputs stream over both hwdge queues (sync + gpsimd) in dependency
    order with 4KB-row packed layouts; output is fp16.
  - A quarter-density warm-up matmul block rides out the PE pstate ramp
    during the DMA fill without draining the power-credit pool.
"""

import sys

sys.path.insert(0, "/opt/trn_rl_repo")
sys.path.insert(0, "/root/.axon_site")

import numpy as np

B, L, D = 2, 2048, 1024
H = 16                  # total heads
HD = 64                 # head dim
HPC = 4                 # heads per core
NCORES = 8
LC = L // 512           # 512-wide l chunks
KC2 = D // 256          # 256-deep DoubleRow contraction chunks
LT = L // 128           # 128-row l tiles
NWARM = 12              # PE warm-up matmuls

_cache = {}
_EXP_SCALE = [1.0]
_V_SCALE = [1.0]


def _build_nc(causal: bool):
    import contextlib

    import concourse.bass as bass
    import concourse.tile as tile
    from concourse import bacc, mybir

    F32 = mybir.dt.float32
    BF16 = mybir.dt.bfloat16
    FP8 = mybir.dt.float8e4
    F16 = mybir.dt.float16
    EXP = mybir.ActivationFunctionType.Exp
    COPY = mybir.ActivationFunctionType.Copy
    DR = mybir.MatmulPerfMode.DoubleRow

    exp_scale = float(_EXP_SCALE[0])
    v_scale = float(_V_SCALE[0])

    nc = bacc.Bacc("TRN2", target_bir_lowering=False, debug=False, num_devices=NCORES)

    # x fp8 packed per lc chunk: [128, (kc2*2+plane)*512 + n] (4KB rows);
    # r8 = fp8 residual of x8 in the same scale domain
    x8 = nc.dram_tensor("x8", [4 * 128, 4096], FP8, kind="ExternalInput")
    r8 = nc.dram_tensor("r8", [4 * 128, 4096], FP8, kind="ExternalInput")
    # Q/K DoubleRow weights, one 4KB-row tensor each:
    # col block (variant*8 + kc2*2 + nt)*256 + plane*128 + m
    # variant 0 = straight feature order, 1 = rope-pair-swapped columns
    wq8 = nc.dram_tensor("wq8", [128, 4096], FP8, kind="ExternalInput")
    wk8 = nc.dram_tensor("wk8", [128, 4096], FP8, kind="ExternalInput")
    # V weights fp8 + residual: col block (var*4 + kc2)*512 + plane*256 + v
    wv8 = nc.dram_tensor("wv8", [128, 4096], FP8, kind="ExternalInput")
    wo = nc.dram_tensor("wo", [256, D], BF16, kind="ExternalInput")
    cosr = nc.dram_tensor("cosr", [32, L], BF16, kind="ExternalInput")
    srot = nc.dram_tensor("srot", [64, L], BF16, kind="ExternalInput")
    mk4 = nc.dram_tensor("mk4", [128, 128], BF16, kind="ExternalInput")
    outT = nc.dram_tensor("outT", [D, L], F16, kind="ExternalOutput")

    with tile.TileContext(nc) as tc, \
         nc.allow_low_precision(reason="fp8/bf16 matmul pipeline by design"), \
         contextlib.ExitStack() as ctx:
        p_w8 = ctx.enter_context(tc.tile_pool(name="p_w8", bufs=3))
        p_wo = ctx.enter_context(tc.tile_pool(name="p_wo", bufs=2))
        p_const = ctx.enter_context(tc.tile_pool(name="p_const", bufs=3))
        p_x8 = ctx.enter_context(tc.tile_pool(name="p_x8", bufs=8))
        p_qk = ctx.enter_context(tc.tile_pool(name="p_qk", bufs=4))
        p_yt = ctx.enter_context(tc.tile_pool(name="p_yt", bufs=2))
        p_v = ctx.enter_context(tc.tile_pool(name="p_v", bufs=16))
        p_pt = ctx.enter_context(tc.tile_pool(name="p_pt", bufs=17))
        p_tmp = ctx.enter_context(tc.tile_pool(name="p_tmp", bufs=6))
        p_zs = ctx.enter_context(tc.tile_pool(name="p_zs", bufs=4))
        p_zb = ctx.enter_context(tc.tile_pool(name="p_zb", bufs=3))
        p_oc = ctx.enter_context(tc.tile_pool(name="p_oc", bufs=8))
        p_wu = ctx.enter_context(tc.tile_pool(name="p_wu", bufs=1))
        pp = ctx.enter_context(tc.tile_pool(name="pp", bufs=2, space="PSUM"))
        pst = ctx.enter_context(tc.tile_pool(name="pst", bufs=2, space="PSUM"))
        pso = ctx.enter_context(tc.tile_pool(name="pso", bufs=2, space="PSUM"))

        # ---- warm-up: keep PE busy during DMA fill (pstate ramp) --------
        wu = p_wu.tile([128, 512], BF16, tag="wu")
        nc.vector.memset(wu[:, :], 0.125)
        wu_ps = pp.tile([128, 512], F32, tag="pp")
        for _ in range(NWARM):
            nc.tensor.matmul(wu_ps[0:32, :], wu[:, 0:32], wu[:, :],
                             start=True, stop=True)

        # ---- input DMAs over both hwdge queues, dependency order.
        # One dma_start binds ONE DMA engine (~25GB/s), so every load is
        # split into partition strips that run on engines in parallel. ----
        x8_sb, r8_sb = {}, {}

        def load_xr(src, dst, lc, eng):
            t = p_x8.tile([128, 8, 512], FP8, tag="x8",
                          name=f"{src.name}_{lc}")
            flat = t[:, :, :].rearrange("p b n -> p (b n)")
            for s in range(4):
                eng.dma_start(
                    out=flat[32 * s:32 * (s + 1), :],
                    in_=src.ap()[lc * 128 + 32 * s:lc * 128 + 32 * (s + 1), :])
            dst[lc] = t

        def load_w8(dram, eng):
            t = p_w8.tile([128, 4096], FP8, tag="w8")
            for s in range(4):
                eng.dma_start(out=t[32 * s:32 * (s + 1), :],
                              in_=dram.ap()[32 * s:32 * (s + 1), :])
            return t

        # sync queue: Q path, then x/r chunks
        cos_t = p_const.tile([128, L], BF16, tag="const")
        for s in range(2):
            nc.sync.dma_start(out=cos_t[16 * s:16 * (s + 1), :],
                              in_=cosr.ap()[16 * s:16 * (s + 1), :])
        wq8_t = load_w8(wq8, nc.sync)
        load_xr(x8, x8_sb, 0, nc.sync)
        load_xr(r8, r8_sb, 0, nc.sync)
        load_xr(x8, x8_sb, 1, nc.sync)
        load_xr(r8, r8_sb, 1, nc.sync)
        load_xr(x8, x8_sb, 2, nc.sync)
        load_xr(r8, r8_sb, 2, nc.sync)
        # gpsimd queue: K path, V weights, late x chunks
        mk_t = p_const.tile([128, 128], BF16, tag="tri")
        nc.gpsimd.dma_start(out=mk_t, in_=mk4.ap())
        srot_t = p_const.tile([128, L], BF16, tag="const")
        for s in range(2):
            nc.gpsimd.dma_start(out=srot_t[32 * s:32 * (s + 1), :],
                                in_=srot.ap()[32 * s:32 * (s + 1), :])
        wk8_t = load_w8(wk8, nc.gpsimd)
        wv8_t3 = p_w8.tile([128, 16, 256], FP8, tag="w8")
        wv8_flat = wv8_t3[:, :, :].rearrange("p b n -> p (b n)")
        for s in range(4):
            nc.gpsimd.dma_start(out=wv8_flat[32 * s:32 * (s + 1), :],
                                in_=wv8.ap()[32 * s:32 * (s + 1), :])
        wv8_t = wv8_t3
        load_xr(x8, x8_sb, 3, nc.gpsimd)
        load_xr(r8, r8_sb, 3, nc.gpsimd)
        wo_sb = []
        for kc2 in range(2):
            t = p_wo.tile([128, D], BF16, tag="wo")
            for s in range(2):
                nc.gpsimd.dma_start(
                    out=t[64 * s:64 * (s + 1), :],
                    in_=wo.ap()[kc2 * 128 + 64 * s:kc2 * 128 + 64 * (s + 1), :])
            wo_sb.append(t)

        # replicate rope tables to 128 partitions (doubling copies on DVE)
        nc.vector.tensor_copy(cos_t[32:64, :], cos_t[0:32, :])
        nc.vector.tensor_copy(cos_t[64:128, :], cos_t[0:64, :])
        nc.vector.tensor_copy(srot_t[64:128, :], srot_t[0:64, :])

        # persistent activation tiles: bf16 Q^T/K^T, 2 heads per nt tile,
        # rows h*64+u with u<32 = even rotary dims, u>=32 = odd dims
        qt_sb = [p_qk.tile([128, L], BF16, tag="qt", name=f"qt{i}")
                 for i in range(2)]
        kt_sb = [p_qk.tile([128, L], BF16, tag="kt", name=f"kt{i}")
                 for i in range(2)]
        yt_sb = [p_yt.tile([128, L], BF16, tag="yt", name=f"yt{i}")
                 for i in range(2)]
        v_sb = [p_v.tile([128, HPC, 65], BF16, tag="vaug", name=f"vaug{i}")
                for i in range(LT)]
        for lt in range(LT):
            nc.gpsimd.memset(v_sb[lt][:, :, 64:65], 1.0)

        # ---- QK projection (fp8 DoubleRow x2) + RoPE -------------------
        def proj_nt(w_t, trg, lc, nt):
            csl = slice(lc * 512, (lc + 1) * 512)
            ps1 = pp.tile([128, 512], F32, tag="pp")
            ps2 = pp.tile([128, 512], F32, tag="pp")
            for dst, var in ((ps1, 0), (ps2, 1)):
                for kc2 in range(KC2):
                    woff = (var * 8 + kc2 * 2 + nt) * 256
                    nc.tensor.matmul(
                        dst[:, :],
                        w_t[:, woff:woff + 256].rearrange(
                            "p (two m) -> p two m", two=2),
                        x8_sb[lc][:, 2 * kc2:2 * kc2 + 2, :],
                        start=(kc2 == 0), stop=(kc2 == KC2 - 1),
                        perf_mode=DR)
            m1 = p_tmp.tile([128, 512], BF16, tag="tmp")
            nc.vector.tensor_mul(m1[:, :], ps1[:, :], cos_t[:, csl])
            m2 = p_tmp.tile([128, 512], BF16, tag="tmp")
            nc.vector.tensor_mul(m2[:, :], ps2[:, :], srot_t[:, csl])
            nc.vector.tensor_add(trg[nt][:, csl], m1[:, :], m2[:, :])

        def proj_qk(lc, interleave=False):
            if interleave:
                for nt in range(2):
                    proj_nt(wq8_t, qt_sb, lc, nt)
                    proj_nt(wk8_t, kt_sb, lc, nt)
            else:
                for nt in range(2):
                    proj_nt(wq8_t, qt_sb, lc, nt)
                for nt in range(2):
                    proj_nt(wk8_t, kt_sb, lc, nt)

        # ---- V tile (fp8 DoubleRow + first-order residual) -------------
        def v_tile(lt):
            lc, o = lt // 4, (lt % 4) * 128
            ps = pp.tile([128, 256], F32, tag="pp")
            # kc2-outer so consecutive matmuls reuse the same x8 stationary
            steps = [(kc2, src, var) for kc2 in range(KC2)
                     for src, var in ((x8_sb[lc], 0), (x8_sb[lc], 1))]
            steps += [(kc2, r8_sb[lc], 0) for kc2 in range(KC2)]
            for si, (kc2, src, var) in enumerate(steps):
                nc.tensor.matmul(
                    ps[:, :],
                    src[:, 2 * kc2:2 * kc2 + 2, o:o + 128],
                    wv8_t[:, (var * 4 + kc2) * 2:(var * 4 + kc2) * 2 + 2, :],
                    start=(si == 0), stop=(si == len(steps) - 1),
                    perf_mode=DR)
            nc.vector.tensor_scalar_mul(
                v_sb[lt][:, :, 0:64],
                ps[:, :].rearrange("p (h v) -> p h v", h=HPC), v_scale)

        # ---- attention -------------------------------------------------
        pending_yt = []   # deferred normalize muls (DVE must not stall on
                          # the gpsimd broadcast latency)

        def flush_yt():
            while pending_yt:
                oaug, zb, nt, r0, csl = pending_yt.pop(0)
                nc.vector.tensor_mul(yt_sb[nt][r0:r0 + 64, csl],
                                     oaug[0:64, :], zb[:, :])

        def normalize(oaug, nt, r0, csl):
            zs = p_zs.tile([1, 512], F32, tag="zs")
            nc.vector.tensor_copy(zs[0:1, :], oaug[64:65, :])
            zrow = p_zs.tile([1, 512], F32, tag="zrow")
            nc.vector.reciprocal_approx_fast(zrow[0:1, :], zs[0:1, :])
            zb = p_zb.tile([64, 512], F32, tag="zb")
            nc.gpsimd.partition_broadcast(zb[:, :], zrow[0:1, :])
            flush_yt()
            pending_yt.append((oaug, zb, nt, r0, csl))

        def trim(c, j):
            k = j - 4 * c
            return 128 * k if (causal and k >= 0) else 0

        def s_exp(c, h, jp):
            nt, r0 = h // 2, (h % 2) * 64
            st = pst.tile([128, 1024], F32, tag="st")
            for s in range(2):
                j = 2 * jp + s
                t = trim(c, j)
                nc.tensor.matmul(
                    st[:, s * 512 + t:(s + 1) * 512],
                    kt_sb[nt][r0:r0 + 64, j * 128:(j + 1) * 128],
                    qt_sb[nt][r0:r0 + 64, c * 512 + t:(c + 1) * 512],
                    start=True, stop=True)
            pt = p_pt.tile([128, 1024], BF16, tag="pt")
            t0 = trim(c, 2 * jp)
            nc.scalar.activation(pt[:, t0:], st[:, t0:], EXP, scale=exp_scale)
            if causal:
                for s in range(2):
                    k = 2 * jp + s - 4 * c
                    if k >= 0:
                        sl = slice(s * 512 + 128 * k, s * 512 + 128 * (k + 1))
                        nc.vector.tensor_mul(pt[:, sl], pt[:, sl], mk_t[:, :])
            return pt

        def att_se(c, h):
            jmax = 4 * c + 3 if causal else LT - 1
            return [(jp, s_exp(c, h, jp)) for jp in range((jmax + 1) // 2)]

        def att_o(c, h, pts):
            nt, r0 = h // 2, (h % 2) * 64
            csl = slice(c * 512, (c + 1) * 512)
            jmax = 4 * c + 3 if causal else LT - 1
            oaug = pso.tile([65, 512], F32, tag="oaug")
            for jp, pt in pts:
                for s in range(2):
                    j = 2 * jp + s
                    t = trim(c, j)
                    nc.tensor.matmul(
                        oaug[:, t:512], v_sb[j][:, h, :],
                        pt[:, s * 512 + t:(s + 1) * 512],
                        start=(j == 0), stop=(j == jmax))
            normalize(oaug, nt, r0, csl)

        def att_full(c, h, lag=2):
            nt, r0 = h // 2, (h % 2) * 64
            csl = slice(c * 512, (c + 1) * 512)
            jmax = 4 * c + 3 if causal else LT - 1
            oaug = pso.tile([65, 512], F32, tag="oaug")

            def emit_o(jp, pt):
                for s in range(2):
                    j = 2 * jp + s
                    t = trim(c, j)
                    nc.tensor.matmul(
                        oaug[:, t:512], v_sb[j][:, h, :],
                        pt[:, s * 512 + t:(s + 1) * 512],
                        start=(j == 0), stop=(j == jmax))

            lagq = []
            for jp in range((jmax + 1) // 2):
                lagq.append((jp, s_exp(c, h, jp)))
                if len(lagq) > lag:
                    emit_o(*lagq.pop(0))
            for args in lagq:
                emit_o(*args)
            normalize(oaug, nt, r0, csl)

        # ---- output projection pieces ----------------------------------
        # (no yt flush needed: wo reads a chunk finished a full phase ago,
        # except the tail call which flushes explicitly)
        def wo_piece(c, ots, tail=False):
            for oi, ot in enumerate(ots):
                ps = pp.tile([128, 512], F32, tag="pp")
                for kc2 in range(2):
                    nc.tensor.matmul(
                        ps[:, :], wo_sb[kc2][:, ot * 128:(ot + 1) * 128],
                        yt_sb[kc2][:, c * 512:(c + 1) * 512],
                        start=(kc2 == 0), stop=(kc2 == 1))
                oc = p_oc.tile([128, 512], F16, tag="oc")
                if tail and oi % 2 == 0:
                    nc.scalar.activation(oc[:, :], ps[:, :], COPY)
                else:
                    nc.vector.tensor_copy(oc[:, :], ps[:, :])
                for s in range(4):
                    nc.sync.dma_start(
                        out=outT.ap()[ot * 128 + 32 * s:
                                      ot * 128 + 32 * (s + 1),
                                      c * 512:(c + 1) * 512],
                        in_=oc[32 * s:32 * (s + 1), :])

        # ---- emission schedule -----------------------------------------
        # The exp (ACT) stream is the critical resource: S/exp blocks run
        # one head ahead of the O-chains, and tensor fill work (V tiles,
        # projections, output pieces) is spread between O-chains so the
        # in-order tensor queue never delays an exp. The chunk's V tiles
        # must all be emitted before its first O-chain; rope projections
        # for chunk c are emitted during phase c-1.
        def att_phase(c, fills):
            fills = list(fills) + [None] * 8
            pts = {0: att_se(c, 0), 1: att_se(c, 1)}
            for h in range(HPC):
                if fills[2 * h] is not None:
                    fills[2 * h]()
                att_o(c, h, pts[h])
                if fills[2 * h + 1] is not None:
                    fills[2 * h + 1]()
                if h + 2 < HPC:
                    pts[h + 2] = att_se(c, h + 2)

        proj_qk(0, interleave=True)

        def f_projq(lc):
            return lambda: [proj_nt(wq8_t, qt_sb, lc, nt) for nt in range(2)]

        def f_projk(lc):
            return lambda: [proj_nt(wk8_t, kt_sb, lc, nt) for nt in range(2)]

        def f_v(l0, n=2):
            return lambda: [v_tile(lt) for lt in range(l0, l0 + n)]

        def f_wo(c, o0):
            return lambda: wo_piece(c, range(o0, o0 + 4))

        att_phase(0, [f_v(0, 4), f_projq(1), None, f_projk(1)])
        att_phase(1, [f_v(4, 4), f_projq(2), None, f_projk(2)])
        att_phase(2, [f_v(8, 4), f_projq(3), None, f_projk(3),
                      f_wo(0, 0), None, f_wo(0, 4)])
        att_phase(3, [f_v(12, 4), f_wo(1, 0), None, f_wo(1, 4),
                      f_wo(2, 0), None, f_wo(2, 4)])
        flush_yt()
        wo_piece(3, range(8), tail=True)

    nc.compile()
    return nc


def _get_nc(causal: bool, exp_scale: float, v_scale: float):
    key = ("causal" if causal else "dense",
           round(float(exp_scale), 18), round(float(v_scale), 12))
    if key not in _cache:
        _EXP_SCALE[0] = float(exp_scale)
        _V_SCALE[0] = float(v_scale)
        _cache[key] = _build_nc(causal)
    return _cache[key]


def _rope_np(x):
    d, s = x.shape[-1], x.shape[-2]
    ts = np.arange(0, d, 2, dtype=np.float32)
    inv = 10000.0 ** (-ts / d)
    grid = np.arange(s, dtype=np.float32)[:, None] * inv[None, :]
    sin = np.repeat(np.sin(grid), 2, axis=-1)
    cos = np.repeat(np.cos(grid), 2, axis=-1)
    x1, x2 = x[..., ::2], x[..., 1::2]
    xs = np.stack([-x2, x1], axis=-1).reshape(x.shape)
    return x * cos + xs * sin


def _reference_np(x, mask, Wq, Wk, Wv, Wo):
    b, l, d = x.shape
    h, k_sz = H, D // H
    split = lambda t: t.reshape(b, l, h, k_sz).transpose(0, 2, 1, 3)
    q = split((x @ Wq) / np.sqrt(np.float32(d)))
    q = _rope_np(q)
    k = _rope_np(split(x @ Wk))
    v = split(x @ Wv)
    logits = np.einsum("bhik,bhjk->bhij", q, k) + mask
    m = logits.max(axis=-1, keepdims=True)
    p = np.exp(logits - m)
    a = p / p.sum(axis=-1, keepdims=True)
    y = np.einsum("bhij,bhjv->bhiv", a, v)
    y = y.transpose(0, 2, 1, 3).reshape(b, l, d)
    return (y @ Wo).astype(np.float32)


def _spectral_norm(w, iters=12):
    rng = np.random.default_rng(0)
    v = rng.standard_normal(w.shape[1]).astype(np.float32)
    for _ in range(iters):
        u = w @ v
        u /= (np.linalg.norm(u) + 1e-30)
        v = w.T @ u
        nv = np.linalg.norm(v)
        v /= (nv + 1e-30)
    return float(nv)


def _host_consts():
    inv = 10000.0 ** (-np.arange(0, HD, 2, dtype=np.float32) / HD)
    grid = np.arange(L, dtype=np.float32)[None, :] * inv[:, None]   # [32, L]
    cos32 = np.cos(grid).astype(np.float32)
    sin32 = np.sin(grid).astype(np.float32)
    # srot rows u: u<32 (even-dim rows) get -sin, u>=32 get +sin
    srot64 = np.ascontiguousarray(np.concatenate([-sin32, sin32], axis=0))
    tri = (np.arange(128)[None, :] >= np.arange(128)[:, None]).astype(np.float32)
    return np.ascontiguousarray(cos32), srot64, np.ascontiguousarray(tri)


def _pack_dr_w(Wc, Wc_s, scale, e4):
    """Wc/Wc_s [1024, 256] (straight / pair-swapped col orders) ->
    [128, 4096] fp8: col block (var*8 + kc2*2 + nt)*256 + plane*128 + m."""
    out = np.empty((128, 4096), np.float32)
    for var, W in ((0, Wc), (1, Wc_s)):
        for kc2 in range(KC2):
            for nt in range(2):
                off = (var * 8 + kc2 * 2 + nt) * 256
                blk = W[kc2 * 256:(kc2 + 1) * 256,
                        nt * 128:(nt + 1) * 128]  # [256, 128]
                out[:, off:off + 128] = blk[0:128]
                out[:, off + 128:off + 256] = blk[128:256]
    return np.clip(out * scale, -240, 240).astype(e4)


def _pack_xr(xTs, e4):
    """xTs [1024, 2048] (already scaled) -> (x8, r8) packed [4*128, 4096]:
    row lc*128+p, col (kc2*2+plane)*512 + n."""
    pk = np.empty((4 * 128, 4096), np.float32)
    for lc in range(LC):
        lsl = slice(lc * 512, (lc + 1) * 512)
        for kc2 in range(KC2):
            for pl in range(2):
                c0 = (kc2 * 2 + pl) * 512
                pk[lc * 128:(lc + 1) * 128, c0:c0 + 512] = \
                    xTs[kc2 * 256 + pl * 128:kc2 * 256 + (pl + 1) * 128, lsl]
    x8 = np.clip(pk, -240, 240).astype(e4)
    r8 = np.clip(pk - x8.astype(np.float32), -240, 240).astype(e4)
    return x8, r8


def _pack_wv(Wvg, scale, e4):
    """Wvg [1024, 256] -> [128, 4096] fp8 (+residual):
    col block (var*4 + kc2)*512 + plane*256 + v."""
    base = np.empty((128, 2048), np.float32)
    for kc2 in range(KC2):
        for pl in range(2):
            c0 = (kc2 * 2 + pl) * 256
            base[:, c0:c0 + 256] = \
                Wvg[kc2 * 256 + pl * 128:kc2 * 256 + (pl + 1) * 128, :]
    base *= scale
    w8 = np.clip(base, -240, 240).astype(e4)
    wr8 = np.clip(base - w8.astype(np.float32), -240, 240).astype(e4)
    out = np.empty((128, 4096), e4)
    out[:, 0:2048] = w8
    out[:, 2048:4096] = wr8
    return out


def _make_in_maps(x, Wq, Wk, Wv, Wo):
    import ml_dtypes
    bf16 = ml_dtypes.bfloat16
    e4 = ml_dtypes.float8_e4m3

    cos32, srot64, tri = _host_consts()

    sx = 240.0 / max(float(np.abs(x).max()), 1e-30)
    swq = 240.0 / max(float(np.abs(Wq).max()), 1e-30)
    swk = 240.0 / max(float(np.abs(Wk).max()), 1e-30)
    swv = 240.0 / max(float(np.abs(Wv).max()), 1e-30)
    # rope tables are plain cos/srot shared by Q and K; all fp8 scales and
    # q's 1/sqrt(d_model) cancel inside the exp() activation scale
    exp_scale = 1.0 / (sx * sx * swq * swk * float(np.sqrt(np.float32(D))))
    v_scale = 1.0 / (sx * swv)

    cosr = np.ascontiguousarray(cos32.astype(bf16))
    srot = np.ascontiguousarray(srot64.astype(bf16))
    mk4 = tri.astype(bf16)

    in_maps = []
    for core in range(NCORES):
        bi, g = core // 4, core % 4
        xTb = x[bi].T  # [1024, 2048] f32
        x8, r8 = _pack_xr(xTb * sx, e4)

        # feature column orders: straight = per head [even dims | odd dims],
        # swapped = per head [odd dims | even dims] (rope pair partners)
        cols, cols_s = [], []
        for hh in range(HPC):
            base = (g * HPC + hh) * 64
            ev = list(range(base, base + 64, 2))
            od = list(range(base + 1, base + 64, 2))
            cols.extend(ev + od)
            cols_s.extend(od + ev)
        in_maps.append({
            "x8": x8,
            "r8": r8,
            "wq8": _pack_dr_w(Wq[:, cols], Wq[:, cols_s], swq, e4),
            "wk8": _pack_dr_w(Wk[:, cols], Wk[:, cols_s], swk, e4),
            "wv8": _pack_wv(Wv[:, g * 256:(g + 1) * 256], swv, e4),
            "wo": np.ascontiguousarray(
                Wo[g * 256:(g + 1) * 256, :].astype(bf16)),
            "cosr": cosr, "srot": srot, "mk4": mk4,
        })
    return in_maps, exp_scale, v_scale


def kernel(x, mask, Wq, Wk, Wv, Wo):
    from concourse.bass_utils import run_bass_kernel_spmd

    x = np.asarray(x, dtype=np.float32)
    mask = np.asarray(mask, dtype=np.float32)
    Wq = np.asarray(Wq, dtype=np.float32)
    Wk = np.asarray(Wk, dtype=np.float32)
    Wv = np.asarray(Wv, dtype=np.float32)
    Wo = np.asarray(Wo, dtype=np.float32)

    m = mask.reshape(L, L)
    tril = np.tril(np.ones((L, L), dtype=bool))
    visible = m > -1e6
    if np.array_equal(visible, tril) and not m[tril].any():
        causal = True
    else:
        # the emission schedule interleaves V tiles per causal chunk; a
        # dense mask would need all V tiles before the first O chain, so
        # route anything non-causal through the host fallback
        return _reference_np(x, mask, Wq, Wk, Wv, Wo)

    # overflow guard for the no-max-subtraction softmax
    xr = float(np.sqrt((x * x).sum(axis=2).max()))
    bound = (xr * _spectral_norm(Wq) / np.sqrt(D)) * (xr * _spectral_norm(Wk))
    if bound > 60.0:
        return _reference_np(x, mask, Wq, Wk, Wv, Wo)

    in_maps, exp_scale, v_scale = _make_in_maps(x, Wq, Wk, Wv, Wo)
    nc = _get_nc(causal, exp_scale, v_scale)
    res = run_bass_kernel_spmd(nc, in_maps, core_ids=list(range(NCORES)))

    out = np.empty((B, L, D), dtype=np.float32)
    for bi in range(B):
        acc = res.results[bi * 4]["outT"].astype(np.float32)
        for g in range(1, 4):
            acc += res.results[bi * 4 + g]["outT"].astype(np.float32)
        out[bi] = acc.T
    return out


# revision 67
# speedup vs baseline: 1.1271x; 1.1271x over previous
"""Multi-head attention (16 heads, RoPE, causal) for Trainium2, 8 NeuronCores.

Sharding: data-parallel over batch (2) x tensor-parallel over head groups (4),
one (batch, head-group-of-4) pair per core. Each core computes its 4 heads'
attention feature-major and a partial output projection outT [1024, 2048] in
fp16; the host sums the 4 partials per batch and transposes back.

Key design points (vs a straightforward bf16 kernel):
  - Q/K projections run in fp8e4 DoubleRow perf mode (2 contraction planes
    per matmul, 0.5 cycles/col = 4x bf16 MAC rate). RoPE's pair-swap is
    precomputed as a second projection with host-shuffled weight columns, so
    on-chip RoPE is just rot = cos*P + srot*Pswap: two PSUM multiplies and
    one add on DVE per tile, nothing on the scalar engine.
  - V projection is fp8 DoubleRow with first-order error compensation:
    V = x8@wv8 + x8@dwv8 + dx8@wv8 (residuals quantized in the same scale
    domain) - more accurate than a bf16 matmul at under half the cycles,
    and it removes the 4MB bf16 x load entirely.
  - The S^T matmul stays bf16 with K=64 contraction: at half array density
    it does not trip the PE power governor (fp8 S drains the power-credit
    pool and throttles the whole attention phase to a 50% duty cycle).
  - All fp8/bf16 scale factors cancel inside the exp() activation scale
    immediate; the rope tables are plain cos/srot, loaded as [32, L] and
    replicated to 128 partitions with two doubling DVE copies each.
  - Scalar (ACT) engine runs exp() only (~84us of 1/cycle/partition work,
    the critical resource); softmax denominators come from a ones-column in
    V_aug; 1/z is broadcast on gpsimd with the yt scale deferred one head
    so DVE never stalls on the broadcast latency.
  - Inputs stream over both hwdge queues (sync + gpsimd) in dependency
    order with 4KB-row packed layouts; output is fp16.
  - A quarter-density warm-up matmul block rides out the PE pstate ramp
    during the DMA fill without draining the power-credit pool.
"""

import sys

sys.path.insert(0, "/opt/trn_rl_repo")
sys.path.insert(0, "/root/.axon_site")

import numpy as np

B, L, D = 2, 2048, 1024
H = 16                  # total heads
HD = 64                 # head dim
HPC = 4                 # heads per core
NCORES = 8
LC = L // 512           # 512-wide l chunks
KC2 = D // 256          # 256-deep DoubleRow contraction chunks
LT = L // 128           # 128-row l tiles
NWARM = 12              # PE warm-up matmuls

_cache = {}
_EXP_SCALE = [1.0]
_V_SCALE = [1.0]


def _build_nc(causal: bool):
    import contextlib

    import concourse.bass as bass
    import concourse.tile as tile
    from concourse import bacc, mybir

    F32 = mybir.dt.float32
    BF16 = mybir.dt.bfloat16
    FP8 = mybir.dt.float8e4
    F16 = mybir.dt.float16
    EXP = mybir.ActivationFunctionType.Exp
    COPY = mybir.ActivationFunctionType.Copy
    DR = mybir.MatmulPerfMode.DoubleRow

    exp_scale = float(_EXP_SCALE[0])
    v_scale = float(_V_SCALE[0])

    nc = bacc.Bacc("TRN2", target_bir_lowering=False, debug=False, num_devices=NCORES)

    # x fp8 packed per lc chunk: [128, (kc2*2+plane)*512 + n] (4KB rows);
    # r8 = fp8 residual of x8 in the same scale domain
    x8 = nc.dram_tensor("x8", [4 * 128, 4096], FP8, kind="ExternalInput")
    r8 = nc.dram_tensor("r8", [4 * 128, 4096], FP8, kind="ExternalInput")
    # Q/K DoubleRow weights, one 4KB-row tensor each:
    # col block (variant*8 + kc2*2 + nt)*256 + plane*128 + m
    # variant 0 = straight feature order, 1 = rope-pair-swapped columns
    wq8 = nc.dram_tensor("wq8", [128, 4096], FP8, kind="ExternalInput")
    wk8 = nc.dram_tensor("wk8", [128, 4096], FP8, kind="ExternalInput")
    # V weights fp8 + residual: col block (var*4 + kc2)*512 + plane*256 + v
    wv8 = nc.dram_tensor("wv8", [128, 4096], FP8, kind="ExternalInput")
    wo = nc.dram_tensor("wo", [256, D], BF16, kind="ExternalInput")
    cosr = nc.dram_tensor("cosr", [32, L], BF16, kind="ExternalInput")
    srot = nc.dram_tensor("srot", [64, L], BF16, kind="ExternalInput")
    mk4 = nc.dram_tensor("mk4", [128, 128], BF16, kind="ExternalInput")
    outT = nc.dram_tensor("outT", [D, L], F16, kind="ExternalOutput")

    with tile.TileContext(nc) as tc, \
         nc.allow_low_precision(reason="fp8/bf16 matmul pipeline by design"), \
         contextlib.ExitStack() as ctx:
        p_w8 = ctx.enter_context(tc.tile_pool(name="p_w8", bufs=3))
        p_wo = ctx.enter_context(tc.tile_pool(name="p_wo", bufs=2))
        p_const = ctx.enter_context(tc.tile_pool(name="p_const", bufs=3))
        p_x8 = ctx.enter_context(tc.tile_pool(name="p_x8", bufs=8))
        p_qk = ctx.enter_context(tc.tile_pool(name="p_qk", bufs=4))
        p_yt = ctx.enter_context(tc.tile_pool(name="p_yt", bufs=2))
        p_v = ctx.enter_context(tc.tile_pool(name="p_v", bufs=16))
        p_pt = ctx.enter_context(tc.tile_pool(name="p_pt", bufs=17))
        p_tmp = ctx.enter_context(tc.tile_pool(name="p_tmp", bufs=6))
        p_zs = ctx.enter_context(tc.tile_pool(name="p_zs", bufs=4))
        p_zb = ctx.enter_context(tc.tile_pool(name="p_zb", bufs=3))
        p_oc = ctx.enter_context(tc.tile_pool(name="p_oc", bufs=8))
        p_wu = ctx.enter_context(tc.tile_pool(name="p_wu", bufs=1))
        pp = ctx.enter_context(tc.tile_pool(name="pp", bufs=2, space="PSUM"))
        pst = ctx.enter_context(tc.tile_pool(name="pst", bufs=2, space="PSUM"))
        pso = ctx.enter_context(tc.tile_pool(name="pso", bufs=2, space="PSUM"))

        # ---- warm-up: keep PE busy during DMA fill (pstate ramp) --------
        wu = p_wu.tile([128, 512], BF16, tag="wu")
        nc.vector.memset(wu[:, :], 0.125)
        wu_ps = pp.tile([128, 512], F32, tag="pp")
        for _ in range(NWARM):
            nc.tensor.matmul(wu_ps[0:32, :], wu[:, 0:32], wu[:, :],
                             start=True, stop=True)

        # ---- input DMAs over both hwdge queues, dependency order.
        # One dma_start binds ONE DMA engine (~25GB/s), so every load is
        # split into partition strips that run on engines in parallel. ----
        x8_sb, r8_sb = {}, {}

        def load_xr(src, dst, lc, eng):
            t = p_x8.tile([128, 8, 512], FP8, tag="x8",
                          name=f"{src.name}_{lc}")
            flat = t[:, :, :].rearrange("p b n -> p (b n)")
            for s in range(4):
                eng.dma_start(
                    out=flat[32 * s:32 * (s + 1), :],
                    in_=src.ap()[lc * 128 + 32 * s:lc * 128 + 32 * (s + 1), :])
            dst[lc] = t

        def load_w8(dram, eng):
            t = p_w8.tile([128, 4096], FP8, tag="w8")
            for s in range(4):
                eng.dma_start(out=t[32 * s:32 * (s + 1), :],
                              in_=dram.ap()[32 * s:32 * (s + 1), :])
            return t

        # sync queue: Q path, then x/r chunks
        cos_t = p_const.tile([128, L], BF16, tag="const")
        for s in range(2):
            nc.sync.dma_start(out=cos_t[16 * s:16 * (s + 1), :],
                              in_=cosr.ap()[16 * s:16 * (s + 1), :])
        wq8_t = load_w8(wq8, nc.sync)
        load_xr(x8, x8_sb, 0, nc.sync)
        load_xr(r8, r8_sb, 0, nc.sync)
        load_xr(x8, x8_sb, 1, nc.sync)
        load_xr(r8, r8_sb, 1, nc.sync)
        load_xr(x8, x8_sb, 2, nc.sync)
        load_xr(r8, r8_sb, 2, nc.sync)
        # gpsimd queue: K path, V weights, late x chunks
        mk_t = p_const.tile([128, 128], BF16, tag="tri")
        nc.gpsimd.dma_start(out=mk_t, in_=mk4.ap())
        srot_t = p_const.tile([128, L], BF16, tag="const")
        for s in range(2):
            nc.gpsimd.dma_start(out=srot_t[32 * s:32 * (s + 1), :],
                                in_=srot.ap()[32 * s:32 * (s + 1), :])
        wk8_t = load_w8(wk8, nc.gpsimd)
        wv8_t3 = p_w8.tile([128, 16, 256], FP8, tag="w8")
        wv8_flat = wv8_t3[:, :, :].rearrange("p b n -> p (b n)")
        for s in range(4):
            nc.gpsimd.dma_start(out=wv8_flat[32 * s:32 * (s + 1), :],
                                in_=wv8.ap()[32 * s:32 * (s + 1), :])
        wv8_t = wv8_t3
        load_xr(x8, x8_sb, 3, nc.gpsimd)
        load_xr(r8, r8_sb, 3, nc.gpsimd)
        wo_sb = []
        for kc2 in range(2):
            t = p_wo.tile([128, D], BF16, tag="wo")
            for s in range(2):
                nc.gpsimd.dma_start(
                    out=t[64 * s:64 * (s + 1), :],
                    in_=wo.ap()[kc2 * 128 + 64 * s:kc2 * 128 + 64 * (s + 1), :])
            wo_sb.append(t)

        # replicate rope tables to 128 partitions (doubling copies on DVE)
        nc.vector.tensor_copy(cos_t[32:64, :], cos_t[0:32, :])
        nc.vector.tensor_copy(cos_t[64:128, :], cos_t[0:64, :])
        nc.vector.tensor_copy(srot_t[64:128, :], srot_t[0:64, :])

        # persistent activation tiles: bf16 Q^T/K^T, 2 heads per nt tile,
        # rows h*64+u with u<32 = even rotary dims, u>=32 = odd dims
        qt_sb = [p_qk.tile([128, L], BF16, tag="qt", name=f"qt{i}")
                 for i in range(2)]
        kt_sb = [p_qk.tile([128, L], BF16, tag="kt", name=f"kt{i}")
                 for i in range(2)]
        yt_sb = [p_yt.tile([128, L], BF16, tag="yt", name=f"yt{i}")
                 for i in range(2)]
        v_sb = [p_v.tile([128, HPC, 65], BF16, tag="vaug", name=f"vaug{i}")
                for i in range(LT)]
        for lt in range(LT):
            nc.gpsimd.memset(v_sb[lt][:, :, 64:65], 1.0)

        # ---- QK projection (fp8 DoubleRow x2) + RoPE -------------------
        def proj_nt(w_t, trg, lc, nt):
            csl = slice(lc * 512, (lc + 1) * 512)
            ps1 = pp.tile([128, 512], F32, tag="pp")
            ps2 = pp.tile([128, 512], F32, tag="pp")
            for dst, var in ((ps1, 0), (ps2, 1)):
                for kc2 in range(KC2):
                    woff = (var * 8 + kc2 * 2 + nt) * 256
                    nc.tensor.matmul(
                        dst[:, :],
                        w_t[:, woff:woff + 256].rearrange(
                            "p (two m) -> p two m", two=2),
                        x8_sb[lc][:, 2 * kc2:2 * kc2 + 2, :],
                        start=(kc2 == 0), stop=(kc2 == KC2 - 1),
                        perf_mode=DR)
            m1 = p_tmp.tile([128, 512], BF16, tag="tmp")
            nc.vector.tensor_mul(m1[:, :], ps1[:, :], cos_t[:, csl])
            m2 = p_tmp.tile([128, 512], BF16, tag="tmp")
            nc.vector.tensor_mul(m2[:, :], ps2[:, :], srot_t[:, csl])
            nc.vector.tensor_add(trg[nt][:, csl], m1[:, :], m2[:, :])

        def proj_qk(lc, interleave=False):
            if interleave:
                for nt in range(2):
                    proj_nt(wq8_t, qt_sb, lc, nt)
                    proj_nt(wk8_t, kt_sb, lc, nt)
            else:
                for nt in range(2):
                    proj_nt(wq8_t, qt_sb, lc, nt)
                for nt in range(2):
                    proj_nt(wk8_t, kt_sb, lc, nt)

        # ---- V tile (fp8 DoubleRow + first-order residual) -------------
        def v_tile(lt):
            lc, o = lt // 4, (lt % 4) * 128
            ps = pp.tile([128, 256], F32, tag="pp")
            # kc2-outer so consecutive matmuls reuse the same x8 stationary
            steps = [(kc2, src, var) for kc2 in range(KC2)
                     for src, var in ((x8_sb[lc], 0), (x8_sb[lc], 1))]
            steps += [(kc2, r8_sb[lc], 0) for kc2 in range(KC2)]
            for si, (kc2, src, var) in enumerate(steps):
                nc.tensor.matmul(
                    ps[:, :],
                    src[:, 2 * kc2:2 * kc2 + 2, o:o + 128],
                    wv8_t[:, (var * 4 + kc2) * 2:(var * 4 + kc2) * 2 + 2, :],
                    start=(si == 0), stop=(si == len(steps) - 1),
                    perf_mode=DR)
            nc.vector.tensor_scalar_mul(
                v_sb[lt][:, :, 0:64],
                ps[:, :].rearrange("p (h v) -> p h v", h=HPC), v_scale)

        # ---- attention -------------------------------------------------
        pending_yt = []   # deferred normalize muls (DVE must not stall on
                          # the gpsimd broadcast latency)

        def flush_yt():
            while pending_yt:
                oaug, zb, nt, r0, csl = pending_yt.pop(0)
                nc.vector.tensor_mul(yt_sb[nt][r0:r0 + 64, csl],
                                     oaug[0:64, :], zb[:, :])

        def normalize(oaug, nt, r0, csl):
            zs = p_zs.tile([1, 512], F32, tag="zs")
            nc.vector.tensor_copy(zs[0:1, :], oaug[64:65, :])
            zrow = p_zs.tile([1, 512], F32, tag="zrow")
            nc.vector.reciprocal_approx_fast(zrow[0:1, :], zs[0:1, :])
            zb = p_zb.tile([64, 512], F32, tag="zb")
            nc.gpsimd.partition_broadcast(zb[:, :], zrow[0:1, :])
            flush_yt()
            pending_yt.append((oaug, zb, nt, r0, csl))

        def trim(c, j):
            k = j - 4 * c
            return 128 * k if (causal and k >= 0) else 0

        def s_exp(c, h, jp):
            nt, r0 = h // 2, (h % 2) * 64
            st = pst.tile([128, 1024], F32, tag="st")
            for s in range(2):
                j = 2 * jp + s
                t = trim(c, j)
                nc.tensor.matmul(
                    st[:, s * 512 + t:(s + 1) * 512],
                    kt_sb[nt][r0:r0 + 64, j * 128:(j + 1) * 128],
                    qt_sb[nt][r0:r0 + 64, c * 512 + t:(c + 1) * 512],
                    start=True, stop=True)
            pt = p_pt.tile([128, 1024], BF16, tag="pt")
            t0 = trim(c, 2 * jp)
            nc.scalar.activation(pt[:, t0:], st[:, t0:], EXP, scale=exp_scale)
            if causal:
                for s in range(2):
                    k = 2 * jp + s - 4 * c
                    if k >= 0:
                        sl = slice(s * 512 + 128 * k, s * 512 + 128 * (k + 1))
                        nc.vector.tensor_mul(pt[:, sl], pt[:, sl], mk_t[:, :])
            return pt

        def att_se(c, h):
            jmax = 4 * c + 3 if causal else LT - 1
            return [(jp, s_exp(c, h, jp)) for jp in range((jmax + 1) // 2)]

        def att_o(c, h, pts):
            nt, r0 = h // 2, (h % 2) * 64
            csl = slice(c * 512, (c + 1) * 512)
            jmax = 4 * c + 3 if causal else LT - 1
            oaug = pso.tile([65, 512], F32, tag="oaug")
            for jp, pt in pts:
                for s in range(2):
                    j = 2 * jp + s
                    t = trim(c, j)
                    nc.tensor.matmul(
                        oaug[:, t:512], v_sb[j][:, h, :],
                        pt[:, s * 512 + t:(s + 1) * 512],
                        start=(j == 0), stop=(j == jmax))
            normalize(oaug, nt, r0, csl)

        def att_full(c, h, lag=2):
            nt, r0 = h // 2, (h % 2) * 64
            csl = slice(c * 512, (c + 1) * 512)
            jmax = 4 * c + 3 if causal else LT - 1
            oaug = pso.tile([65, 512], F32, tag="oaug")

            def emit_o(jp, pt):
                for s in range(2):
                    j = 2 * jp + s
                    t = trim(c, j)
                    nc.tensor.matmul(
                        oaug[:, t:512], v_sb[j][:, h, :],
                        pt[:, s * 512 + t:(s + 1) * 512],
                        start=(j == 0), stop=(j == jmax))

            lagq = []
            for jp in range((jmax + 1) // 2):
                lagq.append((jp, s_exp(c, h, jp)))
                if len(lagq) > lag:
                    emit_o(*lagq.pop(0))
            for args in lagq:
                emit_o(*args)
            normalize(oaug, nt, r0, csl)

        # ---- output projection pieces ----------------------------------
        # (no yt flush needed: wo reads a chunk finished a full phase ago,
        # except the tail call which flushes explicitly)
        def wo_piece(c, ots, tail=False):
            for oi, ot in enumerate(ots):
                ps = pp.tile([128, 512], F32, tag="pp")
                for kc2 in range(2):
                    nc.tensor.matmul(
                        ps[:, :], wo_sb[kc2][:, ot * 128:(ot + 1) * 128],
                        yt_sb[kc2][:, c * 512:(c + 1) * 512],
                        start=(kc2 == 0), stop=(kc2 == 1))
                oc = p_oc.tile([128, 512], F16, tag="oc")
                if tail and oi % 2 == 0:
                    nc.scalar.activation(oc[:, :], ps[:, :], COPY)
                else:
                    nc.vector.tensor_copy(oc[:, :], ps[:, :])
                for s in range(2):
                    nc.sync.dma_start(
                        out=outT.ap()[ot * 128 + 64 * s:
                                      ot * 128 + 64 * (s + 1),
                                      c * 512:(c + 1) * 512],
                        in_=oc[64 * s:64 * (s + 1), :])

        # ---- emission schedule -----------------------------------------
        # The exp (ACT) stream is the critical resource: S/exp blocks run
        # one head ahead of the O-chains, and tensor fill work (V tiles,
        # projections, output pieces) is spread between O-chains so the
        # in-order tensor queue never delays an exp. The chunk's V tiles
        # must all be emitted before its first O-chain; rope projections
        # for chunk c are emitted during phase c-1.
        def att_phase(c, fills):
            fills = list(fills) + [None] * 8
            pts = {0: att_se(c, 0), 1: att_se(c, 1)}
            for h in range(HPC):
                if fills[2 * h] is not None:
                    fills[2 * h]()
                att_o(c, h, pts[h])
                if fills[2 * h + 1] is not None:
                    fills[2 * h + 1]()
                if h + 2 < HPC:
                    pts[h + 2] = att_se(c, h + 2)

        proj_qk(0, interleave=True)

        def f_projq(lc):
            return lambda: [proj_nt(wq8_t, qt_sb, lc, nt) for nt in range(2)]

        def f_projk(lc):
            return lambda: [proj_nt(wk8_t, kt_sb, lc, nt) for nt in range(2)]

        def f_v(l0, n=2):
            return lambda: [v_tile(lt) for lt in range(l0, l0 + n)]

        def f_wo(c, o0):
            return lambda: wo_piece(c, range(o0, o0 + 4))

        att_phase(0, [f_v(0, 4), f_projq(1), None, f_projk(1)])
        att_phase(1, [f_v(4, 4), f_projq(2), None, f_projk(2)])
        att_phase(2, [f_v(8, 4), f_projq(3), None, f_projk(3),
                      f_wo(0, 0), None, f_wo(0, 4)])
        att_phase(3, [f_v(12, 4), f_wo(1, 0), None, f_wo(1, 4),
                      f_wo(2, 0), None, f_wo(2, 4)])
        flush_yt()
        wo_piece(3, range(8), tail=True)

    nc.compile()
    return nc


def _get_nc(causal: bool, exp_scale: float, v_scale: float):
    key = ("causal" if causal else "dense",
           round(float(exp_scale), 18), round(float(v_scale), 12))
    if key not in _cache:
        _EXP_SCALE[0] = float(exp_scale)
        _V_SCALE[0] = float(v_scale)
        _cache[key] = _build_nc(causal)
    return _cache[key]


def _rope_np(x):
    d, s = x.shape[-1], x.shape[-2]
    ts = np.arange(0, d, 2, dtype=np.float32)
    inv = 10000.0 ** (-ts / d)
    grid = np.arange(s, dtype=np.float32)[:, None] * inv[None, :]
    sin = np.repeat(np.sin(grid), 2, axis=-1)
    cos = np.repeat(np.cos(grid), 2, axis=-1)
    x1, x2 = x[..., ::2], x[..., 1::2]
    xs = np.stack([-x2, x1], axis=-1).reshape(x.shape)
    return x * cos + xs * sin


def _reference_np(x, mask, Wq, Wk, Wv, Wo):
    b, l, d = x.shape
    h, k_sz = H, D // H
    split = lambda t: t.reshape(b, l, h, k_sz).transpose(0, 2, 1, 3)
    q = split((x @ Wq) / np.sqrt(np.float32(d)))
    q = _rope_np(q)
    k = _rope_np(split(x @ Wk))
    v = split(x @ Wv)
    logits = np.einsum("bhik,bhjk->bhij", q, k) + mask
    m = logits.max(axis=-1, keepdims=True)
    p = np.exp(logits - m)
    a = p / p.sum(axis=-1, keepdims=True)
    y = np.einsum("bhij,bhjv->bhiv", a, v)
    y = y.transpose(0, 2, 1, 3).reshape(b, l, d)
    return (y @ Wo).astype(np.float32)


def _spectral_norm(w, iters=12):
    rng = np.random.default_rng(0)
    v = rng.standard_normal(w.shape[1]).astype(np.float32)
    for _ in range(iters):
        u = w @ v
        u /= (np.linalg.norm(u) + 1e-30)
        v = w.T @ u
        nv = np.linalg.norm(v)
        v /= (nv + 1e-30)
    return float(nv)


def _host_consts():
    inv = 10000.0 ** (-np.arange(0, HD, 2, dtype=np.float32) / HD)
    grid = np.arange(L, dtype=np.float32)[None, :] * inv[:, None]   # [32, L]
    cos32 = np.cos(grid).astype(np.float32)
    sin32 = np.sin(grid).astype(np.float32)
    # srot rows u: u<32 (even-dim rows) get -sin, u>=32 get +sin
    srot64 = np.ascontiguousarray(np.concatenate([-sin32, sin32], axis=0))
    tri = (np.arange(128)[None, :] >= np.arange(128)[:, None]).astype(np.float32)
    return np.ascontiguousarray(cos32), srot64, np.ascontiguousarray(tri)


def _pack_dr_w(Wc, Wc_s, scale, e4):
    """Wc/Wc_s [1024, 256] (straight / pair-swapped col orders) ->
    [128, 4096] fp8: col block (var*8 + kc2*2 + nt)*256 + plane*128 + m."""
    out = np.empty((128, 4096), np.float32)
    for var, W in ((0, Wc), (1, Wc_s)):
        for kc2 in range(KC2):
            for nt in range(2):
                off = (var * 8 + kc2 * 2 + nt) * 256
                blk = W[kc2 * 256:(kc2 + 1) * 256,
                        nt * 128:(nt + 1) * 128]  # [256, 128]
                out[:, off:off + 128] = blk[0:128]
                out[:, off + 128:off + 256] = blk[128:256]
    return np.clip(out * scale, -240, 240).astype(e4)


def _pack_xr(xTs, e4):
    """xTs [1024, 2048] (already scaled) -> (x8, r8) packed [4*128, 4096]:
    row lc*128+p, col (kc2*2+plane)*512 + n."""
    pk = np.empty((4 * 128, 4096), np.float32)
    for lc in range(LC):
        lsl = slice(lc * 512, (lc + 1) * 512)
        for kc2 in range(KC2):
            for pl in range(2):
                c0 = (kc2 * 2 + pl) * 512
                pk[lc * 128:(lc + 1) * 128, c0:c0 + 512] = \
                    xTs[kc2 * 256 + pl * 128:kc2 * 256 + (pl + 1) * 128, lsl]
    x8 = np.clip(pk, -240, 240).astype(e4)
    r8 = np.clip(pk - x8.astype(np.float32), -240, 240).astype(e4)
    return x8, r8


def _pack_wv(Wvg, scale, e4):
    """Wvg [1024, 256] -> [128, 4096] fp8 (+residual):
    col block (var*4 + kc2)*512 + plane*256 + v."""
    base = np.empty((128, 2048), np.float32)
    for kc2 in range(KC2):
        for pl in range(2):
            c0 = (kc2 * 2 + pl) * 256
            base[:, c0:c0 + 256] = \
                Wvg[kc2 * 256 + pl * 128:kc2 * 256 + (pl + 1) * 128, :]
    base *= scale
    w8 = np.clip(base, -240, 240).astype(e4)
    wr8 = np.clip(base - w8.astype(np.float32), -240, 240).astype(e4)
    out = np.empty((128, 4096), e4)
    out[:, 0:2048] = w8
    out[:, 2048:4096] = wr8
    return out


def _make_in_maps(x, Wq, Wk, Wv, Wo):
    import ml_dtypes
    bf16 = ml_dtypes.bfloat16
    e4 = ml_dtypes.float8_e4m3

    cos32, srot64, tri = _host_consts()

    sx = 240.0 / max(float(np.abs(x).max()), 1e-30)
    swq = 240.0 / max(float(np.abs(Wq).max()), 1e-30)
    swk = 240.0 / max(float(np.abs(Wk).max()), 1e-30)
    swv = 240.0 / max(float(np.abs(Wv).max()), 1e-30)
    # rope tables are plain cos/srot shared by Q and K; all fp8 scales and
    # q's 1/sqrt(d_model) cancel inside the exp() activation scale
    exp_scale = 1.0 / (sx * sx * swq * swk * float(np.sqrt(np.float32(D))))
    v_scale = 1.0 / (sx * swv)

    cosr = np.ascontiguousarray(cos32.astype(bf16))
    srot = np.ascontiguousarray(srot64.astype(bf16))
    mk4 = tri.astype(bf16)

    in_maps = []
    for core in range(NCORES):
        bi, g = core // 4, core % 4
        xTb = x[bi].T  # [1024, 2048] f32
        x8, r8 = _pack_xr(xTb * sx, e4)

        # feature column orders: straight = per head [even dims | odd dims],
        # swapped = per head [odd dims | even dims] (rope pair partners)
        cols, cols_s = [], []
        for hh in range(HPC):
            base = (g * HPC + hh) * 64
            ev = list(range(base, base + 64, 2))
            od = list(range(base + 1, base + 64, 2))
            cols.extend(ev + od)
            cols_s.extend(od + ev)
        in_maps.append({
            "x8": x8,
            "r8": r8,
            "wq8": _pack_dr_w(Wq[:, cols], Wq[:, cols_s], swq, e4),
            "wk8": _pack_dr_w(Wk[:, cols], Wk[:, cols_s], swk, e4),
            "wv8": _pack_wv(Wv[:, g * 256:(g + 1) * 256], swv, e4),
            "wo": np.ascontiguousarray(
                Wo[g * 256:(g + 1) * 256, :].astype(bf16)),
            "cosr": cosr, "srot": srot, "mk4": mk4,
        })
    return in_maps, exp_scale, v_scale


def kernel(x, mask, Wq, Wk, Wv, Wo):
    from concourse.bass_utils import run_bass_kernel_spmd

    x = np.asarray(x, dtype=np.float32)
    mask = np.asarray(mask, dtype=np.float32)
    Wq = np.asarray(Wq, dtype=np.float32)
    Wk = np.asarray(Wk, dtype=np.float32)
    Wv = np.asarray(Wv, dtype=np.float32)
    Wo = np.asarray(Wo, dtype=np.float32)

    m = mask.reshape(L, L)
    tril = np.tril(np.ones((L, L), dtype=bool))
    visible = m > -1e6
    if np.array_equal(visible, tril) and not m[tril].any():
        causal = True
    else:
        # the emission schedule interleaves V tiles per causal chunk; a
        # dense mask would need all V tiles before the first O chain, so
        # route anything non-causal through the host fallback
        return _reference_np(x, mask, Wq, Wk, Wv, Wo)

    # overflow guard for the no-max-subtraction softmax
    xr = float(np.sqrt((x * x).sum(axis=2).max()))
    bound = (xr * _spectral_norm(Wq) / np.sqrt(D)) * (xr * _spectral_norm(Wk))
    if bound > 60.0:
        return _reference_np(x, mask, Wq, Wk, Wv, Wo)

    in_maps, exp_scale, v_scale = _make_in_maps(x, Wq, Wk, Wv, Wo)
    nc = _get_nc(causal, exp_scale, v_scale)
    res = run_bass_kernel_spmd(nc, in_maps, core_ids=list(range(NCORES)))

    out = np.empty((B, L, D), dtype=np.float32)
    for bi in range(B):
        acc = res.results[bi * 4]["outT"].astype(np.float32)
        for g in range(1, 4):
            acc += res.results[bi * 4 + g]["outT"].astype(np.float32)
        out[bi] = acc.T
    return out
